# revision 3
# baseline (speedup 1.0000x reference)
"""GQA attention block (QKV proj + RoPE + KV cache append + softmax attention)
on 8 Trainium2 NeuronCores, tensor-parallel over heads.

Sharding: core c owns q-heads [4c, 4c+4) and kv-head c. Each core computes its
head slice over all tokens; host concatenates the per-core output columns.

start_pos is specialized to 0 (the cache is zero-filled and fully overwritten
by the current 2048 tokens, so keys/values == rope(x@wk), x@wv).

Schedule: phase A = batch-0 projections (ot-major, 2+2 psum banks);
phase B = batch-1 projections (ot-major, 1 psum bank) interleaved with
batch-0 attention so the scalar-engine exp hides under proj matmuls;
phase C = batch-1 attention (scalar-bound), with batch-1's V transposes
slipped into its PE slack. Inputs are host-cast to bf16; the output is
written [dv, seq]-major and permuted on the host.
"""

import sys

sys.path.insert(0, "/opt/trn_rl_repo")

import ml_dtypes
import numpy as np

import concourse.bass as bass
import concourse.tile as tile
from concourse import bacc, mybir
from concourse.bass_utils import run_bass_kernel_spmd
from concourse.masks import make_identity

F32 = mybir.dt.float32
BF16 = mybir.dt.bfloat16

B, S, D = 2, 2048, 4096
HQ, HKV, HD = 32, 8, 128
NCORES = 8
HPC = HQ // NCORES          # q heads per core
QDIM = HPC * HD             # per-core q output dim (512)
TOK = B * S                 # 4096 tokens across both batches
KCH = D // 128              # 32 contraction chunks of 128
PCH = 8                     # projection token chunks
PCW = TOK // PCH            # 512 tokens per chunk
SCH = 4                     # s-chunks per batch in attention
SCW = S // SCH              # 512
NTT = S // 128              # 16 key tiles per batch
SCALE = 1.0 / float(np.sqrt(HD))
OT_ORDER = [HPC + 1, HPC] + list(range(HPC))  # V, K, q0..q3

LAST_EXEC_NS = None


def _build_program():
    nc = bacc.Bacc("TRN2", target_bir_lowering=False, debug=False,
                   num_devices=NCORES)

    xt = nc.declare_dram_parameter("xt", [D, TOK], BF16, isOutput=False)
    wq = nc.declare_dram_parameter("wq", [D, QDIM], BF16, isOutput=False)
    wk = nc.declare_dram_parameter("wk", [D, HD], BF16, isOutput=False)
    wv = nc.declare_dram_parameter("wv", [D, HD], BF16, isOutput=False)
    cc = nc.declare_dram_parameter("cc", [128, S], BF16, isOutput=False)
    ss = nc.declare_dram_parameter("ss", [128, S], BF16, isOutput=False)
    # out[b, h, dv, s]; host permutes to [b, s, h*HD+dv]
    out = nc.declare_dram_parameter("out", [B, HPC, HD, S], F32, isOutput=True)

    with tile.TileContext(nc) as tc:
        pers_cm = tc.tile_pool(name="pers", bufs=1)
        pers = pers_cm.__enter__()

        ccs = pers.tile([128, S], BF16)
        sss = pers.tile([128, S], BF16)
        qTb = pers.tile([128, HPC, TOK], BF16)   # [d, head, tok]
        kTb = pers.tile([128, TOK], BF16)        # [d, tok]
        vTb = pers.tile([128, TOK], BF16)        # [dv, tok]
        vtok = pers.tile([128, B * NTT, HD], BF16)  # [t, (b,tt), dv]
        id_bf = pers.tile([128, 128], BF16)
        ones128 = pers.tile([128, 128], BF16)
        wqb = pers.tile([128, KCH, QDIM], BF16)
        wkb = pers.tile([128, KCH, HD], BF16)
        wvb = pers.tile([128, KCH, HD], BF16)

        sb_cm = [tc.tile_pool(name="xTp", bufs=32),
                 tc.tile_pool(name="rope", bufs=2),
                 tc.tile_pool(name="expp", bufs=17),
                 tc.tile_pool(name="trep", bufs=2),
                 tc.tile_pool(name="fin", bufs=2)]
        xTp, ropep, expp, trep, finp = [cm.__enter__() for cm in sb_cm]

        def x_dma(pc):
            tiles = []
            for kc in range(KCH):
                xT = xTp.tile([128, PCW], BF16, tag="xT", name="xT")
                nc.sync.dma_start(
                    out=xT,
                    in_=xt[kc * 128:(kc + 1) * 128, pc * PCW:(pc + 1) * PCW])
                tiles.append(xT)
            return tiles

        x_tiles = {0: x_dma(0)}
        # weights in first-use order: V, K, then Q
        for wsrc, wdst in ((wv, wvb), (wk, wkb), (wq, wqb)):
            for kc in range(KCH):
                nc.gpsimd.dma_start(
                    out=wdst[:, kc, :], in_=wsrc[kc * 128:(kc + 1) * 128, :])
        nc.gpsimd.dma_start(out=ccs, in_=cc[:])
        nc.gpsimd.dma_start(out=sss, in_=ss[:])
        make_identity(nc, id_bf)
        nc.vector.memset(ones128, 1.0)

        def lhsT_for(ot, kc):
            if ot < HPC:
                return wqb[:, kc, ot * 128:(ot + 1) * 128]
            if ot == HPC:
                return wkb[:, kc, :]
            return wvb[:, kc, :]

        def proj_group(pool, pc, ot):
            """One output-column group of the projection for token chunk pc:
            32 accumulating matmuls + rope/copy epilogue."""
            ps = pool.tile([128, PCW], F32, tag="pj", name="pj")
            for kc in range(KCH):
                nc.tensor.matmul(
                    ps, lhsT_for(ot, kc), x_tiles[pc][kc],
                    start=(kc == 0), stop=(kc == KCH - 1))
            tok_sl = bass.ds(pc * PCW, PCW)
            c_sl = bass.ds((pc % (PCH // 2)) * PCW, PCW)
            if ot < HPC + 1:  # rope for q heads and k
                t1 = ropep.tile([128, PCW], F32, tag="t1", name="t1")
                t2 = ropep.tile([128, PCW], F32, tag="t2", name="t2")
                swp = ropep.tile([128, PCW], F32, tag="swp", name="swp")
                nc.vector.tensor_mul(t1, ps, ccs[:, c_sl])
                # pair-partner swap: cross-partition-base copies
                nc.scalar.copy(swp[0:64], ps[64:128])
                nc.scalar.copy(swp[64:128], ps[0:64])
                nc.vector.tensor_mul(t2, swp, sss[:, c_sl])
                dst = qTb[:, ot, tok_sl] if ot < HPC else kTb[:, tok_sl]
                nc.vector.tensor_add(dst, t1, t2)
            else:
                nc.scalar.copy(vTb[:, tok_sl], ps)

        def vt_transpose(pool, tag, tt, copy_eng):
            """vTb token-tile tt -> token-major vtok via PE transpose."""
            pt = pool.tile([128, 128], BF16, tag=tag, name="pt")
            nc.tensor.transpose(pt, vTb[:, tt * 128:(tt + 1) * 128], id_bf)
            copy_eng.tensor_copy(vtok[:, tt, :], pt)

        # ---------------- phase A: batch-0 projections ----------------
        with tc.tile_pool(name="ppA", bufs=2, space="PSUM") as ppA:
            for pc in range(PCH // 2):
                if pc + 1 < PCH:
                    x_tiles[pc + 1] = x_dma(pc + 1)
                for ot in OT_ORDER:
                    proj_group(ppA, pc, ot)
                    # previous chunk's V transposes, hidden behind this
                    # chunk's first matmul group
                    if ot == HPC + 1 and pc > 0:
                        for i in range(PCW // 128):
                            vt_transpose(ppA, "vt",
                                         (pc - 1) * (PCW // 128) + i,
                                         nc.vector)
            for i in range(PCW // 128):
                vt_transpose(ppA, "vt",
                             (PCH // 2 - 1) * (PCW // 128) + i, nc.vector)

        # ---------------- phases B & C ----------------
        with (
            tc.tile_pool(name="ppB", bufs=1, space="PSUM") as ppB,
            tc.tile_pool(name="psS", bufs=2, space="PSUM") as psS,
            tc.tile_pool(name="psO", bufs=2, space="PSUM") as psO,
            tc.tile_pool(name="psM", bufs=1, space="PSUM") as psM,
        ):
            def attn_scores(b, h, sc, vt_queue=None):
                """scores -> exp (PE + ACT front half of a chunk)."""
                q_rhs = qTb[:, h, bass.ds(b * S + sc * SCW, SCW)]
                exps = []
                for g in range(NTT // 2):
                    pS = psS.tile([128, 2 * SCW], F32, tag="S", name="pS")
                    for j in range(2):
                        tt = 2 * g + j
                        nc.tensor.matmul(
                            pS[:, j * SCW:(j + 1) * SCW],
                            kTb[:, b * S + tt * 128:b * S + (tt + 1) * 128],
                            q_rhs, start=True, stop=True)
                    eS = expp.tile([128, 2 * SCW], BF16, tag="e", name="eS")
                    nc.scalar.activation(
                        out=eS, in_=pS,
                        func=mybir.ActivationFunctionType.Exp,
                        scale=SCALE)
                    exps.append(eS)
                    if vt_queue:
                        vt_transpose(ppB, "pj", vt_queue.pop(0), nc.vector)
                return (b, h, sc, exps)

            def attn_av(state):
                """AV matmuls + denominator (one chunk behind scores)."""
                b, h, sc, exps = state
                po = psO.tile([128, SCW], F32, tag="o", name="po")
                for tt in range(NTT):
                    e_rhs = exps[tt // 2][:, (tt % 2) * SCW:
                                          (tt % 2 + 1) * SCW]
                    nc.tensor.matmul(
                        po, vtok[:, b * NTT + tt, :], e_rhs,
                        start=(tt == 0), stop=(tt == NTT - 1))
                # denominator: 4-level DVE tree in dependency order, then a
                # single all-ones matmul reduces partitions
                lvl0, lvl1, lvl2 = [], [], []

                def fold(src, dst_list, tag, g):
                    p = trep.tile([128, SCW], BF16, tag=tag, name="p")
                    nc.vector.tensor_add(p, src[2 * g], src[2 * g + 1])
                    dst_list.append(p)

                for g in range(NTT // 2):
                    p0 = trep.tile([128, SCW], BF16, tag="tr0", name="p0")
                    nc.vector.tensor_add(
                        p0, exps[g][:, 0:SCW], exps[g][:, SCW:2 * SCW])
                    lvl0.append(p0)
                    if g % 2 == 1:
                        fold(lvl0, lvl1, "tr1", g // 2)
                    if g == 3 or g == 7:
                        fold(lvl1, lvl2, "tr2", g // 4)
                den = trep.tile([128, SCW], BF16, tag="tr3", name="den")
                nc.vector.tensor_add(den, lvl2[0], lvl2[1])
                pden = psM.tile([128, SCW], F32, tag="m", name="pden")
                nc.tensor.matmul(pden, ones128, den, start=True, stop=True)
                recip = finp.tile([128, SCW], F32, tag="recip", name="recip")
                nc.vector.reciprocal_approx_fast(out=recip, in_=pden)
                return (b, h, sc, po, recip)

            def attn_tail(state):
                """normalize -> DMA out (one chunk late so PE rolls on)."""
                b, h, sc, po, recip = state
                osb = finp.tile([128, SCW], F32, tag="osb", name="osb")
                nc.vector.tensor_mul(osb, po, recip)
                nc.gpsimd.dma_start(
                    out=out[b, h, :, sc * SCW:(sc + 1) * SCW], in_=osb)

            # software-pipelined chunk driver shared by phases B and C
            pipe = {"sc": None, "av": None}

            def attn_chunk(key, vt_queue=None):
                st = attn_scores(*key, vt_queue=vt_queue)
                if pipe["sc"] is not None:
                    av2 = attn_av(pipe["sc"])
                    if pipe["av"] is not None:
                        attn_tail(pipe["av"])
                    pipe["av"] = av2
                pipe["sc"] = st

            chunks_b0 = [(0, h, sc) for h in range(HPC) for sc in range(SCH)]
            chunks_b1 = [(1, h, sc) for h in range(HPC) for sc in range(SCH)]

            # phase B: batch-1 proj (ot-major) + batch-0 attention
            ci = 0
            for pc in range(PCH // 2, PCH):
                if pc + 1 < PCH:
                    x_tiles[pc + 1] = x_dma(pc + 1)
                for ot in OT_ORDER:
                    proj_group(ppB, pc, ot)
                    if ot in (HPC + 1, 0, 1, 2):
                        attn_chunk(chunks_b0[ci])
                        ci += 1

            # phase C: batch-1 attention; the first two chunks' PE slack
            # absorbs batch-1's V transposes
            vt_queue = list(range(NTT, 2 * NTT))
            for key in chunks_b1:
                attn_chunk(key, vt_queue=vt_queue if vt_queue else None)
            attn_flush_state = pipe["sc"]
            av2 = attn_av(attn_flush_state)
            if pipe["av"] is not None:
                attn_tail(pipe["av"])
            attn_tail(av2)

        for cm in reversed(sb_cm):
            cm.__exit__(None, None, None)
        pers_cm.__exit__(None, None, None)

    nc.finalize()
    return nc


_ROPE_PERM = np.concatenate(
    [np.arange(0, HD, 2), np.arange(1, HD, 2)])  # even dims then odd dims


def _shard_inputs(x, wq, wk, wv, freqs_cos, freqs_sin):
    bf = ml_dtypes.bfloat16
    x_flat = np.ascontiguousarray(x.astype(np.float32).reshape(TOK, D))
    xT = np.ascontiguousarray(x_flat.T.astype(bf))                # [D, TOK]
    cosT = freqs_cos.T.astype(np.float32)                         # [64, S]
    sinT = freqs_sin.T.astype(np.float32)
    cc = np.ascontiguousarray(
        np.concatenate([cosT, cosT], axis=0).astype(bf))          # [128, S]
    ssm = np.ascontiguousarray(
        np.concatenate([-sinT, sinT], axis=0).astype(bf))

    in_maps = []
    for c in range(NCORES):
        wq_c = np.empty((D, QDIM), bf)
        for j in range(HPC):
            h = HPC * c + j
            wq_c[:, j * HD:(j + 1) * HD] = wq[:, h * HD + _ROPE_PERM].astype(bf)
        wk_c = np.ascontiguousarray(wk[:, c * HD + _ROPE_PERM].astype(bf))
        wv_c = np.ascontiguousarray(wv[:, c * HD:(c + 1) * HD].astype(bf))
        in_maps.append({
            "xt": xT,
            "wq": wq_c, "wk": wk_c, "wv": wv_c,
            "cc": cc, "ss": ssm,
        })
    return in_maps


def kernel(x, wq, wk, wv, cache_k, cache_v, freqs_cos, freqs_sin, start_pos):
    global LAST_EXEC_NS
    x = np.asarray(x)
    wq, wk, wv = np.asarray(wq), np.asarray(wk), np.asarray(wv)
    freqs_cos, freqs_sin = np.asarray(freqs_cos), np.asarray(freqs_sin)
    assert int(start_pos) == 0, "kernel specialized for start_pos == 0"
    assert x.shape == (B, S, D)

    nc = _build_program()
    in_maps = _shard_inputs(x, wq, wk, wv, freqs_cos, freqs_sin)
    res = run_bass_kernel_spmd(nc, in_maps, core_ids=list(range(NCORES)))
    LAST_EXEC_NS = res.exec_time_ns

    full = np.empty((B, S, HQ * HD), np.float32)
    for c in range(NCORES):
        # res[c]["out"]: [B, HPC, HD, S] -> [B, S, HPC*HD]
        oc = np.asarray(res.results[c]["out"])
        full[:, :, c * QDIM:(c + 1) * QDIM] = (
            oc.transpose(0, 3, 1, 2).reshape(B, S, QDIM))
    return full
